# revision 1
# baseline (speedup 1.0000x reference)
"""Trainium2 Bass kernel for nn_FGNet (gnn_message_passing).

Strategy
--------
Per-edge weights are gathers from tiny tables (169 edge types), so edges are
sorted by type id and processed in uniform 256-edge blocks (one id per block,
padded; 2 segments x 128 edges).  Device math per block:

    t_h   = relu(W_id.T @ feats_h + b_id)        h = 0,1
    p_h,i = prod_{j != i} t_h,j                  products, 3 wide DVE muls
    msg_i = W2_id,i.T @ [p_0,i | p_1,i]          N=256 matmul per i
    (the second bias b2 is linear in the segment-sum -> folded to the host)

Matmuls run in float32r (single-pass fp32, ~1.5e-4 relmax, 4x faster than
fp32's 2-pass mode).  HW constraints found empirically on this stack:
  - f32r matmuls need K=128 (K=64 silently returns zeros)
  - matmul *input* partition offsets crash the runtime (NRT unrecoverable)
  - f32r + nonzero *output* partition offset emits tile_position -> invalid ISA
  - DVE memset of an f32r AP is invalid ISA (memset via an f32 bitcast)
  - every instruction gets at most ONE sync wait; Bacc.finalize()'s
    generate_event_semaphores pass splits multi-waits legally
So the transform runs K=128 with zero-padded stationary weights [W;0]/[0;W]
(zeros baked host-side into the packed block), and the second matmul keeps
all outputs at partition offset 0: per block ps2 is [64, 3, 256]; block pairs
are merged into a [128, 768] tile (GPSIMD does the cross-partition move for
odd blocks) so the store DMA uses all 128 partitions / 16 DMA ports.

Packed input per block (pk: [128 partitions, 832 f32r columns]):
    cols   0:384  feats   p = 64*h + l, col = i*128 + e
    cols 384:512  wA = [W; 0]
    cols 512:640  wB = [0; W]
    cols 640:832  ho      row r, col i*64 + l = ho_params[i, id, r, l]
Output msgs[q, 64*parity + e, i*256 + h*128... ] -- see _postprocess.

Host side (vectorized numpy): id computation, sort, feature gather, packing,
unpermute, b2 bias add and the final segment-sum into node_msg.
"""

import numpy as np

_BLK = 256          # edge slots per block (2 segments x 128)
_SEG = 128
_FCOLS = 832        # packed pk columns per block
_NCORES = 8

_prog_cache = {}


def _build_program(B):
    """Build the SPMD device program for B blocks per core (B even)."""
    import concourse.mybir as mybir
    import concourse.tile as tile
    from concourse import bacc

    F32 = mybir.dt.float32
    F32R = mybir.dt.float32r
    Relu = mybir.ActivationFunctionType.Relu
    Copy = mybir.ActivationFunctionType.Copy

    assert B % 2 == 0
    PB = B // 2

    nc = bacc.Bacc()
    pk = nc.declare_dram_parameter("pk", [B, 128, _FCOLS], F32R, isOutput=False)
    bia = nc.declare_dram_parameter("bia", [128, B], F32, isOutput=False)
    msgs = nc.declare_dram_parameter("msgs", [B, 64, 768], F32, isOutput=True)

    with tile.TileContext(nc) as tc:
        with (
            tc.tile_pool(name="const", bufs=1) as const,
            tc.tile_pool(name="work", bufs=4) as work,
            tc.tile_pool(name="psum", bufs=2, space="PSUM") as psum,
        ):
            bt = const.tile([128, B], F32, name="bt")
            nc.sync.dma_start(out=bt[:], in_=bia[:])

            for b in range(B):
                pkt = work.tile([128, _FCOLS], F32R, name="pkt", tag="pkt")
                nc.sync.dma_start(out=pkt[:], in_=pk[b])

                # transform: one psum tile, both segments
                ps1 = psum.tile([128, 2, 512], F32, name="ps1", tag="ps1")
                nc.tensor.matmul(out=ps1[:, 0, 0:384], lhsT=pkt[:, 384:512],
                                 rhs=pkt[:, 0:384], start=True, stop=True)
                nc.tensor.matmul(out=ps1[:, 1, 0:384], lhsT=pkt[:, 512:640],
                                 rhs=pkt[:, 0:384], start=True, stop=True)

                t = work.tile([128, 2, 384], F32, name="t", tag="t")
                nc.scalar.activation(out=t[:], in_=ps1[:, :, 0:384],
                                     func=Relu, bias=bt[:, b:b + 1],
                                     scale=1.0)

                # products: p[:, i, h, :] = prod_{j != i} t_h,j
                p = work.tile([128, 3, 2, 128], F32R, name="p", tag="p")
                nc.vector.tensor_mul(out=p[:, 0], in0=t[:, :, 128:256],
                                     in1=t[:, :, 256:384])
                nc.vector.tensor_mul(out=p[:, 1], in0=t[:, :, 0:128],
                                     in1=t[:, :, 256:384])
                nc.vector.tensor_mul(out=p[:, 2], in0=t[:, :, 0:128],
                                     in1=t[:, :, 128:256])

                # second matmul: msg_i = ho_i.T @ [p_0,i | p_1,i], N=256
                ps2 = psum.tile([64, 3, 256], F32, name="ps2", tag="ps2")
                for i in range(3):
                    nc.tensor.matmul(
                        out=ps2[:, i, :],
                        lhsT=pkt[:, 640 + 64 * i:640 + 64 * (i + 1)],
                        rhs=p[:, i].rearrange("r h e -> r (h e)"),
                        start=True, stop=True,
                    )

                ps2f = ps2[:].rearrange("l i he -> l (i he)")
                m = work.tile([64, 768], F32, name="m", tag="m")
                if b % 2 == 0:
                    nc.vector.tensor_copy(out=m[:], in_=ps2f)
                else:
                    nc.scalar.activation(out=m[:], in_=ps2f, func=Copy,
                                         bias=0.0, scale=1.0)
                nc.sync.dma_start(out=msgs[b], in_=m[:])
    nc.finalize()
    return nc


def _get_program(B):
    if B not in _prog_cache:
        _prog_cache[B] = _build_program(B)
    return _prog_cache[B]


def _prepare(x, nodes, fact, params, bias_p, ho_params, ho_bias):
    """Host-side: sort by id, build per-block packed arrays."""
    N, L = nodes.shape
    E = fact.shape[0]
    R = params.shape[2]
    NP = params.shape[0]           # 169
    MA = int(round(NP ** 0.5))     # 13

    ids = (x[fact[:, 0], 1] * MA + x[fact[:, 0], 2]).astype(np.int64)   # [E]
    perm = np.argsort(ids, kind="stable")
    ids_s = ids[perm]
    fact_s = fact[perm].astype(np.int64)                                 # [E,3]

    counts = np.bincount(ids_s, minlength=NP)                            # [NP]
    nblk = (counts + _BLK - 1) // _BLK                                   # [NP]
    blk_ids = np.repeat(np.arange(NP), nblk)                             # [NB]
    NB = int(blk_ids.shape[0])
    B = (NB + _NCORES - 1) // _NCORES
    if B % 2:
        B += 1
    NB8 = B * _NCORES
    blk_ids = np.concatenate([blk_ids, np.zeros(NB8 - NB, np.int64)])

    # slot -> sorted-edge-position map (-1 = padding)
    padded = nblk * _BLK
    pad_off = np.concatenate([[0], np.cumsum(padded)])
    off = np.concatenate([[0], np.cumsum(counts)])
    total = int(pad_off[-1])
    t_of = np.repeat(np.arange(NP), padded)
    jloc = np.arange(total) - pad_off[t_of]
    src = np.where(jloc < counts[t_of], off[t_of] + jloc, -1)
    src = np.concatenate([src, np.full(NB8 * _BLK - total, -1, np.int64)])
    valid = src >= 0

    # gather features per slot
    nf = nodes[fact_s]                                                   # [E,3,L]
    featp = np.zeros((NB8 * _BLK, 3, L), np.float32)
    featp[valid] = nf[src[valid]]

    # pack feats + [W;0] + [0;W] + ho
    pk = np.zeros((NB8, 128, _FCOLS), np.float32)
    pk[:, :, 0:384] = (
        featp.reshape(NB8, 2, _SEG, 3, L).transpose(0, 1, 4, 3, 2)
        .reshape(NB8, 128, 384)
    )
    W = params[blk_ids].astype(np.float32)                               # [NB8,L,R]
    pk[:, 0:64, 384:512] = W
    pk[:, 64:128, 512:640] = W
    pk[:, :, 640:832] = (
        ho_params[:, blk_ids].astype(np.float32).transpose(1, 2, 0, 3)
        .reshape(NB8, R, 3 * L)
    )

    biasT = bias_p[blk_ids, 0].astype(np.float32)                        # [NB8,R]
    biasT = biasT.reshape(_NCORES, B, R).transpose(0, 2, 1)              # [8,R,B]

    return dict(pk=pk, biasT=np.ascontiguousarray(biasT), B=B, NB8=NB8,
                src=src, valid=valid, fact_s=fact_s, ids_s=ids_s,
                N=N, E=E, L=L)


def _postprocess(msgs_all, prep, ho_bias):
    """Decode per-slot messages, add host-side b2, segment-sum into node_msg."""
    NB8, N, E, L = prep["NB8"], prep["N"], prep["E"], prep["L"]
    src, valid, fact_s, ids_s = prep["src"], prep["valid"], prep["fact_s"], prep["ids_s"]
    # msgs_all [NB8, 64, 768]: row = l, col = i*256 + h*128 + e
    slots = (
        msgs_all.reshape(NB8, 64, 3, 2, _SEG).transpose(0, 3, 4, 2, 1)
        .reshape(NB8 * _BLK, 3, 64)
    )
    msg_e = np.empty((E, 3, L), np.float32)
    msg_e[src[valid]] = slots[valid]

    # fold in the second bias (linear in the segment-sum)
    msg_e += ho_bias[:, ids_s, 0].astype(np.float32).transpose(1, 0, 2)  # [E,3,L]

    idx_all = fact_s.T.reshape(-1)                                       # [3E]
    val_all = msg_e.transpose(1, 0, 2).reshape(-1, L)                    # [3E,L]
    order = np.argsort(idx_all, kind="stable")
    idx_sorted = idx_all[order]
    val_sorted = val_all[order]
    uniq, starts = np.unique(idx_sorted, return_index=True)
    sums = np.add.reduceat(val_sorted, starts, axis=0)
    out = np.zeros((N, L), np.float32)
    out[uniq] = sums
    return out


def _run_device(prep, trace=False, trace_kwargs=None):
    from concourse.bass_utils import run_bass_kernel_spmd

    B = prep["B"]
    nc = _get_program(B)
    in_maps = []
    for c in range(_NCORES):
        in_maps.append({
            "pk": prep["pk"][c * B:(c + 1) * B],
            "bia": prep["biasT"][c],
        })
    kwargs = {}
    if trace:
        kwargs["trace"] = True
        if trace_kwargs:
            kwargs.update(trace_kwargs)
    res = run_bass_kernel_spmd(nc, in_maps, list(range(_NCORES)), **kwargs)
    msgs_all = np.concatenate([res.results[c]["msgs"] for c in range(_NCORES)],
                              axis=0)
    return msgs_all, res


def kernel(x, nodes, fact, fact_dim, params, bias_p, ho_params, ho_bias,
           _trace=False, _trace_kwargs=None):
    x = np.asarray(x)
    nodes = np.asarray(nodes, dtype=np.float32)
    fact = np.asarray(fact)
    params = np.asarray(params)
    bias_p = np.asarray(bias_p)
    ho_params = np.asarray(ho_params)
    ho_bias = np.asarray(ho_bias)

    prep = _prepare(x, nodes, fact, params, bias_p, ho_params, ho_bias)
    msgs_all, res = _run_device(prep, trace=_trace, trace_kwargs=_trace_kwargs)
    out = _postprocess(msgs_all, prep, ho_bias)
    kernel.last_results = res
    return out



# revision 2
# speedup vs baseline: 1.6263x; 1.6263x over previous
"""Trainium2 Bass kernel for nn_FGNet (gnn_message_passing) — v3 (fp16 + PE tiling).

Strategy
--------
Edges sorted by type id, uniform 256-edge blocks (one id per block, padded;
2 segments x 128 edges), processed in PAIRS (blocks 2q, 2q+1).  All device
tensors fp16 (PSUM accumulation stays f32); rel err ~1e-3 vs the 2e-2 gate.

Per pair q, with block a on SBUF partitions 0:64 and block b on 64:128:

    mm1 (K=64, row-tiled): T(0,0) computes W_a.T @ feats_a while T(64,0)
        computes W_b.T @ feats_b concurrently; N=384 halves (PSUM-bank aligned)
    relu+bias (ACT): psum f32 -> t fp16, one 3D-AP instruction per block
    products (DVE): p_i = t_j * t_k, fp16 at 2 elem/cycle, 3 per block
    mm2 (M=64, col-tiled): T(0,0) writes msg_a to psum partitions 0:64 while
        T(0,64) writes msg_b to 64:128 -> natural [128,768] pair layout
    copy (DVE): psum f32 -> m fp16 [128,768], one instruction per pair
    (second bias b2 is linear in the segment-sum -> folded to the host)

HW constraints baked in (validated on this stack by mb2.py):
  - matmul output must lie within one 2KB PSUM bank (N<=512 f32, no crossing)
  - DMA cannot touch PSUM; GPSIMD cannot touch PSUM
  - PE tiling via tile_position works for fp16 (row tiles need lhsT/rhs on
    the matching SBUF partition half; col tiles write psum partition halves)
  - f32r needs K=128 (v2 legacy); fp16 K=64 is fine

Packed inputs per pair (host-side numpy):
    fk  [128, 768] fp16   partition 64c+l, col = seg*384 + i*128 + e
    wk  [128, 128] fp16   rows 0:64 = W_a, rows 64:128 = W_b
    hot [128, 384] fp16   col 192c + 64i + l = ho_params[i, id_c, r, l]
    bia [128, B]   f32    column 2q+c = bias_p[id_c]
Output msgs[q] [128, 768] fp16: partition 64c+l, col = i*256 + seg*128 + e.

Host side: id computation, sort, feature gather, packing, unpermute, b2 bias
add and the final segment-sum into node_msg.
"""

import numpy as np

_BLK = 256          # edge slots per block (2 segments x 128)
_SEG = 128
_NCORES = 8

_prog_cache = {}


def _build_program(P):
    """Build the SPMD device program for P block-pairs per core."""
    import concourse.mybir as mybir
    import concourse.tile as tile
    from concourse import bacc

    F32 = mybir.dt.float32
    F16 = mybir.dt.float16
    Relu = mybir.ActivationFunctionType.Relu

    B = 2 * P
    nc = bacc.Bacc()
    fk = nc.declare_dram_parameter("fk", [P, 128, 768], F16, isOutput=False)
    wk = nc.declare_dram_parameter("wk", [P, 128, 128], F16, isOutput=False)
    hot = nc.declare_dram_parameter("hot", [P, 128, 384], F16, isOutput=False)
    bia = nc.declare_dram_parameter("bia", [128, B], F32, isOutput=False)
    msgs = nc.declare_dram_parameter("msgs", [P, 128, 768], F16, isOutput=True)

    with tile.TileContext(nc) as tc:
        with (
            tc.tile_pool(name="const", bufs=1) as const,
            tc.tile_pool(name="work", bufs=4) as work,
            tc.tile_pool(name="psA", bufs=1, space="PSUM") as psA,
            tc.tile_pool(name="psB", bufs=2, space="PSUM") as psB,
        ):
            bt = const.tile([128, B], F32, name="bt")
            nc.sync.dma_start(out=bt[:], in_=bia[:])

            for q in range(P):
                fkt = work.tile([128, 768], F16, name="fkt", tag="fkt")
                wkt = work.tile([128, 128], F16, name="wkt", tag="wkt")
                hts = work.tile([128, 384], F16, name="hts", tag="hts")
                nc.sync.dma_start(out=fkt[:], in_=fk[q])
                nc.sync.dma_start(out=wkt[:], in_=wk[q])
                nc.sync.dma_start(out=hts[:], in_=hot[q])

                # mm1: row-tiled K=64 pair, N=384 bank-aligned halves
                ps_a = psA.tile([128, 2, 512], F32, name="ps_a", tag="ps_a")
                ps_b = psA.tile([128, 2, 512], F32, name="ps_b", tag="ps_b")
                for j in range(2):
                    nc.tensor.matmul(
                        out=ps_a[:, j, 0:384], lhsT=wkt[0:64, :],
                        rhs=fkt[0:64, 384 * j:384 * (j + 1)],
                        start=True, stop=True, tile_position=(0, 0))
                    nc.tensor.matmul(
                        out=ps_b[:, j, 0:384], lhsT=wkt[64:128, :],
                        rhs=fkt[64:128, 384 * j:384 * (j + 1)],
                        start=True, stop=True, tile_position=(64, 0))

                ta = work.tile([128, 2, 384], F16, name="ta", tag="ta")
                tb = work.tile([128, 2, 384], F16, name="tb", tag="tb")
                nc.scalar.activation(out=ta[:], in_=ps_a[:, :, 0:384],
                                     func=Relu, bias=bt[:, 2 * q:2 * q + 1],
                                     scale=1.0)
                nc.scalar.activation(out=tb[:], in_=ps_b[:, :, 0:384],
                                     func=Relu, bias=bt[:, 2 * q + 1:2 * q + 2],
                                     scale=1.0)

                # products: p[:, i] = t_j * t_k  (seg-strided 3D APs)
                pa = work.tile([128, 3, 2, 128], F16, name="pa", tag="pa")
                pb = work.tile([128, 3, 2, 128], F16, name="pb", tag="pb")
                for i, (j, k) in enumerate([(1, 2), (0, 2), (0, 1)]):
                    nc.vector.tensor_mul(
                        out=pa[:, i],
                        in0=ta[:, :, 128 * j:128 * (j + 1)],
                        in1=ta[:, :, 128 * k:128 * (k + 1)])
                    nc.vector.tensor_mul(
                        out=pb[:, i],
                        in0=tb[:, :, 128 * j:128 * (j + 1)],
                        in1=tb[:, :, 128 * k:128 * (k + 1)])

                # mm2: col-tiled M=64 pairs into psum partition halves
                ps2 = psB.tile([128, 3, 256], F32, name="ps2", tag="ps2")
                for i in range(3):
                    nc.tensor.matmul(
                        out=ps2[0:64, i, :],
                        lhsT=hts[:, 64 * i:64 * (i + 1)],
                        rhs=pa[:, i].rearrange("r s e -> r (s e)"),
                        start=True, stop=True, tile_position=(0, 0))
                    nc.tensor.matmul(
                        out=ps2[64:128, i, :],
                        lhsT=hts[:, 192 + 64 * i:192 + 64 * (i + 1)],
                        rhs=pb[:, i].rearrange("r s e -> r (s e)"),
                        start=True, stop=True, tile_position=(0, 64))

                m = work.tile([128, 768], F16, name="m", tag="m")
                nc.vector.tensor_copy(
                    out=m[:], in_=ps2[:].rearrange("l i c -> l (i c)"))
                nc.sync.dma_start(out=msgs[q], in_=m[:])
    nc.finalize()
    return nc


def _get_program(P):
    if P not in _prog_cache:
        _prog_cache[P] = _build_program(P)
    return _prog_cache[P]


def _prepare(x, nodes, fact, params, bias_p, ho_params, ho_bias):
    """Host-side: sort by id, build per-pair packed fp16 arrays."""
    N, L = nodes.shape
    E = fact.shape[0]
    R = params.shape[2]
    NP = params.shape[0]           # 169
    MA = int(round(NP ** 0.5))     # 13

    ids = (x[fact[:, 0], 1] * MA + x[fact[:, 0], 2]).astype(np.int64)   # [E]
    perm = np.argsort(ids, kind="stable")
    ids_s = ids[perm]
    fact_s = fact[perm].astype(np.int64)                                 # [E,3]

    counts = np.bincount(ids_s, minlength=NP)                            # [NP]
    nblk = (counts + _BLK - 1) // _BLK                                   # [NP]
    blk_ids = np.repeat(np.arange(NP), nblk)                             # [NB]
    NB = int(blk_ids.shape[0])
    B = (NB + _NCORES - 1) // _NCORES
    if B % 2:
        B += 1
    NB8 = B * _NCORES
    blk_ids = np.concatenate([blk_ids, np.zeros(NB8 - NB, np.int64)])

    # slot -> sorted-edge-position map (-1 = padding)
    padded = nblk * _BLK
    pad_off = np.concatenate([[0], np.cumsum(padded)])
    off = np.concatenate([[0], np.cumsum(counts)])
    total = int(pad_off[-1])
    t_of = np.repeat(np.arange(NP), padded)
    jloc = np.arange(total) - pad_off[t_of]
    src = np.where(jloc < counts[t_of], off[t_of] + jloc, -1)
    src = np.concatenate([src, np.full(NB8 * _BLK - total, -1, np.int64)])
    valid = src >= 0

    # gather features per slot
    nf = nodes[fact_s].astype(np.float16)                                # [E,3,L]
    featp = np.zeros((NB8 * _BLK, 3, L), np.float16)
    featp[valid] = nf[src[valid]]

    NPAIR = NB8 // 2
    # fk: [q, 64c+l, seg*384 + i*128 + e]
    fk = np.ascontiguousarray(
        featp.reshape(NPAIR, 2, 2, _SEG, 3, L)      # q c seg e i l
        .transpose(0, 1, 5, 2, 4, 3)                # q c l seg i e
        .reshape(NPAIR, 128, 768))
    # wk: rows 0:64 = W_a, 64:128 = W_b
    wk = np.ascontiguousarray(
        params[blk_ids].astype(np.float16)          # [NB8, L, R]
        .reshape(NPAIR, 2 * L, R))
    # hot: [q, r, 192c + 64i + l]
    hot = np.ascontiguousarray(
        ho_params[:, blk_ids].astype(np.float16)    # [3, NB8, R, L]
        .transpose(1, 2, 0, 3)                      # NB8 r i l
        .reshape(NPAIR, 2, R, 3 * L)                # q c r (i l)
        .transpose(0, 2, 1, 3)                      # q r c (i l)
        .reshape(NPAIR, R, 384))

    biasT = bias_p[blk_ids, 0].astype(np.float32)                        # [NB8,R]
    biasT = biasT.reshape(_NCORES, B, R).transpose(0, 2, 1)              # [8,R,B]

    return dict(fk=fk, wk=wk, hot=hot, biasT=np.ascontiguousarray(biasT),
                B=B, NB8=NB8, P=B // 2,
                src=src, valid=valid, fact_s=fact_s, ids_s=ids_s,
                N=N, E=E, L=L)


def _postprocess(msgs_all, prep, ho_bias):
    """Decode per-slot messages, add host-side b2, segment-sum into node_msg."""
    NB8, N, E, L = prep["NB8"], prep["N"], prep["E"], prep["L"]
    src, valid, fact_s, ids_s = prep["src"], prep["valid"], prep["fact_s"], prep["ids_s"]
    NPAIR = NB8 // 2
    # msgs_all [NPAIR, 128, 768]: partition 64c+l, col = i*256 + seg*128 + e
    slots = (
        msgs_all.astype(np.float32)
        .reshape(NPAIR, 2, 64, 3, 2, _SEG)          # q c l i seg e
        .transpose(0, 1, 4, 5, 3, 2)                # q c seg e i l
        .reshape(NB8 * _BLK, 3, 64)
    )
    msg_e = np.empty((E, 3, L), np.float32)
    msg_e[src[valid]] = slots[valid]

    # fold in the second bias (linear in the segment-sum)
    msg_e += ho_bias[:, ids_s, 0].astype(np.float32).transpose(1, 0, 2)  # [E,3,L]

    idx_all = fact_s.T.reshape(-1)                                       # [3E]
    val_all = msg_e.transpose(1, 0, 2).reshape(-1, L)                    # [3E,L]
    order = np.argsort(idx_all, kind="stable")
    idx_sorted = idx_all[order]
    val_sorted = val_all[order]
    uniq, starts = np.unique(idx_sorted, return_index=True)
    sums = np.add.reduceat(val_sorted, starts, axis=0)
    out = np.zeros((N, L), np.float32)
    out[uniq] = sums
    return out


def _run_device(prep, trace=False, trace_kwargs=None):
    from concourse.bass_utils import run_bass_kernel_spmd

    P = prep["P"]
    nc = _get_program(P)
    in_maps = []
    for c in range(_NCORES):
        in_maps.append({
            "fk": prep["fk"][c * P:(c + 1) * P],
            "wk": prep["wk"][c * P:(c + 1) * P],
            "hot": prep["hot"][c * P:(c + 1) * P],
            "bia": prep["biasT"][c],
        })
    kwargs = {}
    if trace:
        kwargs["trace"] = True
        if trace_kwargs:
            kwargs.update(trace_kwargs)
    res = run_bass_kernel_spmd(nc, in_maps, list(range(_NCORES)), **kwargs)
    msgs_all = np.concatenate([res.results[c]["msgs"] for c in range(_NCORES)],
                              axis=0)
    return msgs_all, res


def kernel(x, nodes, fact, fact_dim, params, bias_p, ho_params, ho_bias,
           _trace=False, _trace_kwargs=None):
    x = np.asarray(x)
    nodes = np.asarray(nodes, dtype=np.float32)
    fact = np.asarray(fact)
    params = np.asarray(params)
    bias_p = np.asarray(bias_p)
    ho_params = np.asarray(ho_params)
    ho_bias = np.asarray(ho_bias)

    prep = _prepare(x, nodes, fact, params, bias_p, ho_params, ho_bias)
    msgs_all, res = _run_device(prep, trace=_trace, trace_kwargs=_trace_kwargs)
    out = _postprocess(msgs_all, prep, ho_bias)
    kernel.last_results = res
    return out


# revision 7
# speedup vs baseline: 1.6533x; 1.0166x over previous
"""Trainium2 Bass kernel for nn_FGNet (gnn_message_passing) — v3 (fp16 + PE tiling).

Strategy
--------
Edges sorted by type id, uniform 256-edge blocks (one id per block, padded;
2 segments x 128 edges), processed in PAIRS (blocks 2q, 2q+1).  All device
tensors fp16 (PSUM accumulation stays f32); rel err ~1e-3 vs the 2e-2 gate.

Per pair q, with block a on SBUF partitions 0:64 and block b on 64:128:

    mm1 (K=64, row-tiled): T(0,0) computes W_a.T @ feats_a while T(64,0)
        computes W_b.T @ feats_b concurrently; N=384 halves (PSUM-bank aligned)
    relu+bias (ACT): psum f32 -> t fp16, one 3D-AP instruction per block
    products (DVE): p_i = t_j * t_k, fp16 at 2 elem/cycle, 3 per block
    mm2 (M=64, col-tiled): T(0,0) writes msg_a to psum partitions 0:64 while
        T(0,64) writes msg_b to 64:128 -> natural [128,768] pair layout
    copy (DVE): psum f32 -> m fp16 [128,768], one instruction per pair
    (second bias b2 is linear in the segment-sum -> folded to the host)

HW constraints baked in (validated on this stack by mb2.py):
  - matmul output must lie within one 2KB PSUM bank (N<=512 f32, no crossing)
  - DMA cannot touch PSUM; GPSIMD cannot touch PSUM
  - PE tiling via tile_position works for fp16 (row tiles need lhsT/rhs on
    the matching SBUF partition half; col tiles write psum partition halves)
  - f32r needs K=128 (v2 legacy); fp16 K=64 is fine

Packed inputs per pair (host-side numpy):
    fk  [128, 768] fp16   partition 64c+l, col = seg*384 + i*128 + e
    wk  [128, 128] fp16   rows 0:64 = W_a, rows 64:128 = W_b
    hot [128, 384] fp16   col 192c + 64i + l = ho_params[i, id_c, r, l]
    bia [128, B]   f32    column 2q+c = bias_p[id_c]
Output msgs[q] [128, 768] fp16: partition 64c+l, col = i*256 + seg*128 + e.

Host side: id computation, sort, feature gather, packing, unpermute, b2 bias
add and the final segment-sum into node_msg.
"""

import numpy as np

_BLK = 256          # edge slots per block (2 segments x 128)
_SEG = 128
_NCORES = 8

_prog_cache = {}


def _build_program(P):
    """Build the SPMD device program for P block-pairs per core."""
    import concourse.mybir as mybir
    import concourse.tile as tile
    from concourse import bacc

    F32 = mybir.dt.float32
    F16 = mybir.dt.float16
    Relu = mybir.ActivationFunctionType.Relu
    Copy = mybir.ActivationFunctionType.Copy

    B = 2 * P
    nc = bacc.Bacc()
    pkin = nc.declare_dram_parameter("pkin", [P, 128, 1280], F16, isOutput=False)
    bia = nc.declare_dram_parameter("bia", [128, B], F32, isOutput=False)
    msgs = nc.declare_dram_parameter("msgs", [P, 128, 768], F16, isOutput=True)

    with tile.TileContext(nc) as tc:
        with (
            tc.tile_pool(name="const", bufs=1) as const,
            tc.tile_pool(name="work", bufs=6) as work,
            tc.tile_pool(name="psA", bufs=1, space="PSUM") as psA,
            tc.tile_pool(name="psB", bufs=2, space="PSUM") as psB,
        ):
            bt = const.tile([128, B], F32, name="bt")
            nc.sync.dma_start(out=bt[:], in_=bia[:])

            for q in range(P):
                # one merged input DMA per pair: feats | W | ho
                pkt = work.tile([128, 1280], F16, name="pkt", tag="pkt")
                nc.sync.dma_start(out=pkt[:], in_=pkin[q])
                fkt = pkt[:, 0:768]
                wkt = pkt[:, 768:896]

                # mm1: row-tiled K=64 pair, N=384 bank-aligned halves
                ps_a = psA.tile([128, 2, 512], F32, name="ps_a", tag="ps_a")
                ps_b = psA.tile([128, 2, 512], F32, name="ps_b", tag="ps_b")
                for j in range(2):
                    nc.tensor.matmul(
                        out=ps_a[:, j, 0:384], lhsT=wkt[0:64, :],
                        rhs=fkt[0:64, 384 * j:384 * (j + 1)],
                        start=True, stop=True, tile_position=(0, 0))
                    nc.tensor.matmul(
                        out=ps_b[:, j, 0:384], lhsT=wkt[64:128, :],
                        rhs=fkt[64:128, 384 * j:384 * (j + 1)],
                        start=True, stop=True, tile_position=(64, 0))

                ta = work.tile([128, 2, 384], F16, name="ta", tag="ta")
                tb = work.tile([128, 2, 384], F16, name="tb", tag="tb")
                nc.scalar.activation(out=ta[:], in_=ps_a[:, :, 0:384],
                                     func=Relu, bias=bt[:, 2 * q:2 * q + 1],
                                     scale=1.0)
                nc.scalar.activation(out=tb[:], in_=ps_b[:, :, 0:384],
                                     func=Relu, bias=bt[:, 2 * q + 1:2 * q + 2],
                                     scale=1.0)

                # products: p[:, i] = t_j * t_k  (seg-strided 3D APs)
                pa = work.tile([128, 3, 2, 128], F16, name="pa", tag="pa")
                pb = work.tile([128, 3, 2, 128], F16, name="pb", tag="pb")
                for i, (j, k) in enumerate([(1, 2), (0, 2), (0, 1)]):
                    nc.vector.tensor_mul(
                        out=pa[:, i],
                        in0=ta[:, :, 128 * j:128 * (j + 1)],
                        in1=ta[:, :, 128 * k:128 * (k + 1)])
                    nc.vector.tensor_mul(
                        out=pb[:, i],
                        in0=tb[:, :, 128 * j:128 * (j + 1)],
                        in1=tb[:, :, 128 * k:128 * (k + 1)])

                # mm2: col-tiled M=64 pairs into psum partition halves
                ps2 = psB.tile([128, 3, 256], F32, name="ps2", tag="ps2")
                for i in range(3):
                    nc.tensor.matmul(
                        out=ps2[0:64, i, :],
                        lhsT=pkt[:, 896 + 64 * i:896 + 64 * (i + 1)],
                        rhs=pa[:, i].rearrange("r s e -> r (s e)"),
                        start=True, stop=True, tile_position=(0, 0))
                    nc.tensor.matmul(
                        out=ps2[64:128, i, :],
                        lhsT=pkt[:, 1088 + 64 * i:1088 + 64 * (i + 1)],
                        rhs=pb[:, i].rearrange("r s e -> r (s e)"),
                        start=True, stop=True, tile_position=(0, 64))

                # psum evacuation split: scalar takes 1/3, vector 2/3
                ps2f = ps2[:].rearrange("l i c -> l (i c)")
                m = work.tile([128, 768], F16, name="m", tag="m")
                nc.scalar.activation(out=m[:, 0:256], in_=ps2f[:, 0:256],
                                     func=Copy, bias=0.0, scale=1.0)
                nc.vector.tensor_copy(out=m[:, 256:768], in_=ps2f[:, 256:768])
                nc.scalar.dma_start(out=msgs[q], in_=m[:])
    nc.finalize()
    return nc


def _get_program(P):
    if P not in _prog_cache:
        _prog_cache[P] = _build_program(P)
    return _prog_cache[P]


def _prepare(x, nodes, fact, params, bias_p, ho_params, ho_bias):
    """Host-side: sort by id, build per-pair packed fp16 arrays."""
    N, L = nodes.shape
    E = fact.shape[0]
    R = params.shape[2]
    NP = params.shape[0]           # 169
    MA = int(round(NP ** 0.5))     # 13

    ids = (x[fact[:, 0], 1] * MA + x[fact[:, 0], 2]).astype(np.int64)   # [E]
    perm = np.argsort(ids, kind="stable")
    ids_s = ids[perm]
    fact_s = fact[perm].astype(np.int64)                                 # [E,3]

    counts = np.bincount(ids_s, minlength=NP)                            # [NP]
    nblk = (counts + _BLK - 1) // _BLK                                   # [NP]
    blk_ids = np.repeat(np.arange(NP), nblk)                             # [NB]
    NB = int(blk_ids.shape[0])
    B = (NB + _NCORES - 1) // _NCORES
    if B % 2:
        B += 1
    NB8 = B * _NCORES
    blk_ids = np.concatenate([blk_ids, np.zeros(NB8 - NB, np.int64)])

    # slot -> sorted-edge-position map (-1 = padding)
    padded = nblk * _BLK
    pad_off = np.concatenate([[0], np.cumsum(padded)])
    off = np.concatenate([[0], np.cumsum(counts)])
    total = int(pad_off[-1])
    t_of = np.repeat(np.arange(NP), padded)
    jloc = np.arange(total) - pad_off[t_of]
    src = np.where(jloc < counts[t_of], off[t_of] + jloc, -1)
    src = np.concatenate([src, np.full(NB8 * _BLK - total, -1, np.int64)])
    valid = src >= 0

    # gather features per slot
    nf = nodes[fact_s].astype(np.float16)                                # [E,3,L]
    featp = np.zeros((NB8 * _BLK, 3, L), np.float16)
    featp[valid] = nf[src[valid]]

    NPAIR = NB8 // 2
    # pkin: cols 0:768 feats, 768:896 W, 896:1280 ho
    pkin = np.zeros((NPAIR, 128, 1280), np.float16)
    # feats: [q, 64c+l, seg*384 + i*128 + e]
    pkin[:, :, 0:768] = (
        featp.reshape(NPAIR, 2, 2, _SEG, 3, L)      # q c seg e i l
        .transpose(0, 1, 5, 2, 4, 3)                # q c l seg i e
        .reshape(NPAIR, 128, 768))
    # W: rows 0:64 = W_a, 64:128 = W_b
    pkin[:, :, 768:896] = (
        params[blk_ids].astype(np.float16)          # [NB8, L, R]
        .reshape(NPAIR, 2 * L, R))
    # ho: [q, r, 192c + 64i + l]
    pkin[:, :, 896:1280] = (
        ho_params[:, blk_ids].astype(np.float16)    # [3, NB8, R, L]
        .transpose(1, 2, 0, 3)                      # NB8 r i l
        .reshape(NPAIR, 2, R, 3 * L)                # q c r (i l)
        .transpose(0, 2, 1, 3)                      # q r c (i l)
        .reshape(NPAIR, R, 384))

    biasT = bias_p[blk_ids, 0].astype(np.float32)                        # [NB8,R]
    biasT = biasT.reshape(_NCORES, B, R).transpose(0, 2, 1)              # [8,R,B]

    return dict(pkin=pkin, biasT=np.ascontiguousarray(biasT),
                B=B, NB8=NB8, P=B // 2,
                src=src, valid=valid, fact_s=fact_s, ids_s=ids_s,
                N=N, E=E, L=L)


def _postprocess(msgs_all, prep, ho_bias):
    """Decode per-slot messages, add host-side b2, segment-sum into node_msg."""
    NB8, N, E, L = prep["NB8"], prep["N"], prep["E"], prep["L"]
    src, valid, fact_s, ids_s = prep["src"], prep["valid"], prep["fact_s"], prep["ids_s"]
    NPAIR = NB8 // 2
    # msgs_all [NPAIR, 128, 768]: partition 64c+l, col = i*256 + seg*128 + e
    slots = (
        msgs_all.astype(np.float32)
        .reshape(NPAIR, 2, 64, 3, 2, _SEG)          # q c l i seg e
        .transpose(0, 1, 4, 5, 3, 2)                # q c seg e i l
        .reshape(NB8 * _BLK, 3, 64)
    )
    msg_e = np.empty((E, 3, L), np.float32)
    msg_e[src[valid]] = slots[valid]

    # fold in the second bias (linear in the segment-sum)
    msg_e += ho_bias[:, ids_s, 0].astype(np.float32).transpose(1, 0, 2)  # [E,3,L]

    idx_all = fact_s.T.reshape(-1)                                       # [3E]
    val_all = msg_e.transpose(1, 0, 2).reshape(-1, L)                    # [3E,L]
    order = np.argsort(idx_all, kind="stable")
    idx_sorted = idx_all[order]
    val_sorted = val_all[order]
    uniq, starts = np.unique(idx_sorted, return_index=True)
    sums = np.add.reduceat(val_sorted, starts, axis=0)
    out = np.zeros((N, L), np.float32)
    out[uniq] = sums
    return out


def _run_device(prep, trace=False, trace_kwargs=None):
    from concourse.bass_utils import run_bass_kernel_spmd

    P = prep["P"]
    nc = _get_program(P)
    in_maps = []
    for c in range(_NCORES):
        in_maps.append({
            "pkin": prep["pkin"][c * P:(c + 1) * P],
            "bia": prep["biasT"][c],
        })
    kwargs = {}
    if trace:
        kwargs["trace"] = True
        if trace_kwargs:
            kwargs.update(trace_kwargs)
    res = run_bass_kernel_spmd(nc, in_maps, list(range(_NCORES)), **kwargs)
    msgs_all = np.concatenate([res.results[c]["msgs"] for c in range(_NCORES)],
                              axis=0)
    return msgs_all, res


def kernel(x, nodes, fact, fact_dim, params, bias_p, ho_params, ho_bias,
           _trace=False, _trace_kwargs=None):
    x = np.asarray(x)
    nodes = np.asarray(nodes, dtype=np.float32)
    fact = np.asarray(fact)
    params = np.asarray(params)
    bias_p = np.asarray(bias_p)
    ho_params = np.asarray(ho_params)
    ho_bias = np.asarray(ho_bias)

    prep = _prepare(x, nodes, fact, params, bias_p, ho_params, ho_bias)
    msgs_all, res = _run_device(prep, trace=_trace, trace_kwargs=_trace_kwargs)
    out = _postprocess(msgs_all, prep, ho_bias)
    kernel.last_results = res
    return out
